# revision 21
# baseline (speedup 1.0000x reference)
"""Trainium2 Bass kernel for nn_CIFAR10Net LIF conv layer.

Reference computation:
  w' = weight-standardized clip(weight) ; conv2d(x, w', pad=1) over (T*B) frames
  LIF scan over T with state (u, sg) [sm/ss are dead state]:
     sg = (sg + I) * (1 - 1/tau_grad);  u = u + sg
     spike = u >= th ; u, sg *= (1 - spike)
Spikes out: [T, B, 128, 32, 32] f32.

Device mapping (per core, B sharded 4/core over 8 cores):
  - partition dim = Cout (128); free = positions (b, h, w)
  - PE: im2col conv (27-row contraction, 4-way row-packed over b) producing
    cg*I into PSUM bank b, then accumulates cg*Id @ sg (fp32) -> psum = sg'_t
  - DVE custom ops:  sg''_t = select(u+sg' < th, sg', 0)
                     u''_t  = select(u+sg' < th, u+sg', 0)
  - spike: ACT Sign(u'') as int8 (spike <=> u''==0), decoded host-side.
"""

import os
import numpy as np

import concourse.bacc as bacc
import concourse.mybir as mybir
import concourse.dve_ops as dve_ops
from concourse.dve_spec import Spec, Src0, Src1, C0, Zero, select, lower
from concourse.dve_spec import _has_src1
from concourse.dve_uop import DveOpSpec
from concourse.tile import TileContext
from concourse.bass_utils import run_bass_kernel_spmd

# ---------------- constants -------------------------------------------------
T, B, CIN, H, W = 16, 32, 3, 32, 32
COUT, KK = 128, 3
NCORES = 8
BSH = B // NCORES          # 4 batches per core
CG = np.float32(1.0 - 1.0 / 3.5)
NB = 512                   # positions per psum bank (= one batch half)
NHALF = 4 * NB             # 2048 positions per half-step
SPIKE_MODE = os.environ.get("LIF_SPIKE_MODE", "act")  # dve | pool | act
KREPEAT = int(os.environ.get("LIF_KREPEAT", "1"))  # program repetitions (timing)
ABLATE = set(filter(None, os.environ.get("LIF_ABLATE", "").split(",")))  # sim ablations
IDSPLIT = int(os.environ.get("LIF_IDSPLIT", "4"))  # banks 0..IDSPLIT-1 on PE, rest on DVE
POOLS = int(os.environ.get("LIF_POOLS", "448"))  # positions/half offloaded to Pool engine

# ---------------- custom DVE ops -------------------------------------------
_s = Src0 + Src1


def _register_op(name, spec):
    shas = {}
    for ver in ("v3",):
        uops = lower(spec, ver=ver)
        shas[ver] = DveOpSpec(
            name=name, opcode=0, uops=uops, rd1_en=_has_src1(spec)
        ).sha(ver)
    op = dve_ops.DveOp(name, spec, subdim=False, uops_sha=shas)
    for o in dve_ops.OPS:
        if o.name == name:
            return o
    dve_ops.OPS.append(op)
    dve_ops.CUSTOM_DVE_SPECS[name] = spec
    dve_ops._SUB_OPCODE_FOR_NAME[name] = max(dve_ops._SUB_OPCODE_FOR_NAME.values()) + 1
    assert dve_ops._SUB_OPCODE_FOR_NAME[name] < 0x20
    return op


LIF_U = _register_op(
    "LIF_U",
    Spec(
        body=select(_s < C0, _s, Zero),
        reference=lambda in0, in1, s0, s1, imm2: np.where(
            (in0 + in1) < s0, (in0 + in1).astype(np.float32), 0.0
        ).astype(np.float32),
    ),
)
LIF_SG = _register_op(
    "LIF_SG",
    Spec(
        body=select(_s < C0, Src1, Zero),
        reference=lambda in0, in1, s0, s1, imm2: np.where(
            (in0 + in1) < s0, in1, 0.0
        ).astype(np.float32),
    ),
)
# t=0 state update (u = sg = 0): both new states equal select(ps < th, ps, 0).
LIF_P0 = _register_op(
    "LIF_P0",
    Spec(
        body=select(Src0 < C0, Src0, Zero),
        reference=lambda in0, in1, s0, s1, imm2: np.where(
            in0 < s0, in0, 0.0
        ).astype(np.float32),
    ),
)

# ---------------- device kernel builder -------------------------------------
_NC_CACHE = {}


def _build_nc(krepeat=None):
    krepeat = KREPEAT if krepeat is None else krepeat
    key = (SPIKE_MODE, krepeat, IDSPLIT, POOLS, tuple(sorted(ABLATE)))
    if key in _NC_CACHE:
        return _NC_CACHE[key]
    f32 = mybir.dt.float32
    f32r = mybir.dt.float32r
    nc = bacc.Bacc("TRN2", target_bir_lowering=False)

    xpad = nc.dram_tensor("xpad", [T, COUT, 1156], f32, kind="ExternalInput")
    wmat = nc.dram_tensor("wmat", [COUT, COUT], f32, kind="ExternalInput")
    cgid = nc.dram_tensor("cgid", [COUT, COUT], f32r, kind="ExternalInput")
    th = nc.dram_tensor("th", [COUT, 1], f32, kind="ExternalInput")
    spk = nc.dram_tensor(
        "spk", [T, 2, COUT, NHALF],
        mybir.dt.uint8 if SPIKE_MODE != "act" else mybir.dt.int8,
        kind="ExternalOutput",
    )

    with TileContext(nc) as tc, \
         tc.tile_pool(name="const", bufs=1) as cpool, \
         tc.tile_pool(name="state", bufs=1) as spool, \
         tc.tile_pool(name="im", bufs=6) as impool, \
         tc.tile_pool(name="out", bufs=6) as opool, \
         tc.tile_pool(name="vp", bufs=4) as vpool, \
         tc.tile_pool(name="ps", bufs=2, space="PSUM") as ppool:

        w_sb = cpool.tile([COUT, COUT], f32, tag="w")
        id_sb = cpool.tile([COUT, COUT], f32r, tag="id")
        th_sb = cpool.tile([COUT, 1], f32, tag="th")
        nc.sync.dma_start(w_sb[:], wmat[:])
        nc.sync.dma_start(id_sb[:], cgid[:])
        nc.sync.dma_start(th_sb[:], th[:])

        ubuf = [spool.tile([COUT, 2 * NHALF], f32, tag=f"u{i}", name=f"u{i}") for i in range(2)]
        gbuf = [spool.tile([COUT, 2 * NHALF], f32r, tag=f"g{i}", name=f"g{i}") for i in range(2)]
        for _rep in range(krepeat):
          for t in range(T):
              ucur, unext = ubuf[t % 2], ubuf[(t + 1) % 2]
              gcur, gnext = gbuf[t % 2], gbuf[(t + 1) % 2]

              im = impool.tile([COUT, 34, 34], f32, tag="im27")
              if "imdma" in ABLATE:
                  nc.vector.memset(im[:, :, :], 0.0)
              else:
                  eng = nc.sync if t % 2 == 0 else nc.scalar
                  eng.dma_start(im[:, :, :], xpad[t, :, :])

              for half in range(2):
                  lo = half * NHALF
                  ps = ppool.tile([COUT, NHALF], f32, tag="ps")
                  for b in range(BSH) if "conv" not in ABLATE else []:
                      nc.tensor.matmul(
                          ps[:, NB * b : NB * (b + 1)],
                          w_sb[32 * b : 32 * b + 27, :],
                          im[32 * b : 32 * b + 27, 16 * half : 16 * half + 16, 0:32],
                          start=True,
                          stop=(t == 0),
                          tile_position=(32 * b, 0),
                          skip_group_check=True,
                      )
                  for b in (range(BSH) if ("idmm" not in ABLATE and t > 0) else []):
                      if b < IDSPLIT:
                          nc.tensor.matmul(
                              ps[:, NB * b : NB * (b + 1)],
                              id_sb[:],
                              gcur[:, lo + NB * b : lo + NB * (b + 1)],
                              start=False,
                              stop=True,
                              tile_position=(0, 0),
                              skip_group_check=True,
                          )
                      else:
                          nc.vector.scalar_tensor_tensor(
                              ps[:, NB * b : NB * (b + 1)],
                              gcur[:, lo + NB * b : lo + NB * (b + 1)],
                              float(CG),
                              ps[:, NB * b : NB * (b + 1)],
                              mybir.AluOpType.mult,
                              mybir.AluOpType.add,
                          )

                  NDVE = NHALF - (POOLS if t > 0 else 0)
                  if "dve" in ABLATE:
                      nc.vector.memset(gnext[:, lo : lo + NHALF], 0.0)
                      nc.vector.memset(unext[:, lo : lo + NHALF], 0.0)
                  elif t == 0:
                      # u = sg = 0: both updates collapse to sel(ps<th, ps, 0).
                      nc.vector._custom_dve(
                          LIF_P0,
                          out=gnext[:, lo : lo + NHALF],
                          in0=ps[:],
                          s0=th_sb[:],
                      )
                      nc.vector._custom_dve(
                          LIF_P0,
                          out=unext[:, lo : lo + NHALF],
                          in0=ps[:],
                          s0=th_sb[:],
                      )
                  else:
                      # Pool engine handles the tail POOLS positions via
                      # v=u+ps; m=[v<th]; u'=v*m; g'=ps*m (4 stock ops).
                      if POOLS:
                          # GPSIMD can't read PSUM: ACT stages the slice to SBUF.
                          pc = vpool.tile([COUT, POOLS], f32, tag="pc")
                          pv = vpool.tile([COUT, POOLS], f32, tag="pv")
                          pm = vpool.tile([COUT, POOLS], f32, tag="pm")
                          nc.scalar.activation(
                              pc[:], ps[:, NDVE:NHALF],
                              mybir.ActivationFunctionType.Copy)
                          nc.gpsimd.tensor_tensor(
                              pv[:], ucur[:, lo + NDVE : lo + NHALF],
                              pc[:], mybir.AluOpType.add)
                          nc.gpsimd.tensor_scalar(
                              pm[:], pv[:], th_sb[:], None, mybir.AluOpType.is_lt)
                          if t < T - 1:
                              nc.gpsimd.tensor_tensor(
                                  gnext[:, lo + NDVE : lo + NHALF],
                                  pc[:], pm[:], mybir.AluOpType.mult)
                          nc.gpsimd.tensor_tensor(
                              unext[:, lo + NDVE : lo + NHALF],
                              pv[:], pm[:], mybir.AluOpType.mult)
                      if t < T - 1:  # t=T-1 sg state is dead
                          nc.vector._custom_dve(
                              LIF_SG,
                              out=gnext[:, lo : lo + NDVE],
                              in0=ucur[:, lo : lo + NDVE],
                              in1=ps[:, 0:NDVE],
                              s0=th_sb[:],
                          )
                      nc.vector._custom_dve(
                          LIF_U,
                          out=unext[:, lo : lo + NDVE],
                          in0=ucur[:, lo : lo + NDVE],
                          in1=ps[:, 0:NDVE],
                          s0=th_sb[:],
                      )

                  if SPIKE_MODE == "act":
                      st = opool.tile([COUT, NHALF], mybir.dt.int8, tag="spk")
                      nc.scalar.activation(
                          st[:], unext[:, lo : lo + NHALF],
                          mybir.ActivationFunctionType.Sign,
                      )
                  elif SPIKE_MODE == "pool":
                      st = opool.tile([COUT, NHALF], mybir.dt.uint8, tag="spk")
                      nc.gpsimd.tensor_scalar(
                          st[:], unext[:, lo : lo + NHALF], 0.0, None,
                          mybir.AluOpType.is_equal,
                      )
                  else:
                      st = opool.tile([COUT, NHALF], mybir.dt.uint8, tag="spk")
                      nc.vector.tensor_scalar(
                          st[:], unext[:, lo : lo + NHALF], 0.0, None,
                          mybir.AluOpType.is_equal,
                      )
                  if "outdma" not in ABLATE:
                      nc.sync.dma_start(spk[t, half, :, :], st[:])

    nc.finalize()
    _NC_CACHE[key] = nc
    return nc


# ---------------- host side --------------------------------------------------
def _prep_weights(weight, norm_weight, norm_bias):
    w = np.clip(weight.astype(np.float32), -4.0, 4.0)
    flat = w.reshape(COUT, -1)
    mean = flat.mean(axis=1, dtype=np.float32)
    var = flat.var(axis=1, ddof=1, dtype=np.float32)
    scale = (norm_weight.reshape(COUT).astype(np.float32)
             / np.sqrt(var + np.float32(1e-5)))
    w_std = (w - mean[:, None, None, None]) * scale[:, None, None, None] \
        + norm_bias.reshape(COUT, 1, 1, 1).astype(np.float32)
    # wmat[32b + 3*(3dy+dx) + c, co] = cg * w_std[co, c, dy, dx]
    wmat = np.zeros((COUT, COUT), np.float32)
    wk = (CG * w_std).transpose(1, 2, 3, 0)  # [c, dy, dx, co]
    for dy in range(3):
        for dx in range(3):
            r = 3 * (3 * dy + dx)
            for b in range(BSH):
                wmat[32 * b + r : 32 * b + r + 3, :] = wk[:, dy, dx, :]
    return wmat


def kernel(x, weight, norm_weight, norm_bias, threshold, _want_trace=False, _krepeat=None):
    x = np.asarray(x, np.float32)
    nc = _build_nc(_krepeat)
    wmat = _prep_weights(np.asarray(weight), np.asarray(norm_weight),
                         np.asarray(norm_bias))
    cgid = (np.eye(COUT) * CG).astype(np.float32)
    th_h = np.asarray(threshold, np.float32).reshape(COUT, 1)

    xp = np.pad(x, [(0, 0), (0, 0), (0, 0), (1, 1), (1, 1)])  # [T,B,C,34,34]
    # x27[t, 32b + 3*(3dy+dx) + c, f] = xpad[t, b, c].flat[34*dy + dx + f]
    xflat = np.pad(xp.reshape(T, B, CIN * 1156), [(0, 0), (0, 0), (0, 128)])
    x27 = np.zeros((T, B, 32, 1156), np.float32)
    for dy in range(3):
        for dx in range(3):
            for c in range(CIN):
                off = c * 1156 + 34 * dy + dx
                x27[:, :, 3 * (3 * dy + dx) + c, :] = xflat[:, :, off : off + 1156]
    in_maps = []
    for core in range(NCORES):
        xs = np.ascontiguousarray(
            x27[:, core * BSH : (core + 1) * BSH].reshape(T, COUT, 1156)
        )
        in_maps.append({"xpad": xs, "wmat": wmat, "cgid": cgid, "th": th_h})

    res = run_bass_kernel_spmd(
        nc, in_maps, core_ids=list(range(NCORES)), trace=_want_trace
    )

    out = np.empty((T, B, COUT, H, W), np.float32)
    for core in range(NCORES):
        s = res.results[core]["spk"]  # [T, 2, 128, 2048]
        if SPIKE_MODE == "act":
            spikes = (s == 0)
        else:
            spikes = (s != 0)
        # [t, half, co, b, hh, w] -> [t, b, co, 16*half+hh, w]
        spikes = spikes.reshape(T, 2, COUT, BSH, 16, W).transpose(0, 3, 2, 1, 4, 5)
        out[:, core * BSH : (core + 1) * BSH] = spikes.reshape(
            T, BSH, COUT, H, W
        ).astype(np.float32)
    if _want_trace:
        kernel.last_result = res
    return out



# revision 22
# speedup vs baseline: 1.5242x; 1.5242x over previous
"""Trainium2 Bass kernel for nn_CIFAR10Net LIF conv layer.

Reference computation:
  w' = weight-standardized clip(weight) ; conv2d(x, w', pad=1) over (T*B) frames
  LIF scan over T with state (u, sg) [sm/ss are dead state]:
     sg = (sg + I) * (1 - 1/tau_grad);  u = u + sg
     spike = u >= th ; u, sg *= (1 - spike)
Spikes out: [T, B, 128, 32, 32] f32.

Device mapping (per core, B sharded 4/core over 8 cores):
  - partition dim = Cout (128); free = positions (b, h, w)
  - PE: im2col conv (27-row contraction, 4-way row-packed over b, fp32)
    producing cg*I into PSUM, then accumulates cg*Id @ sg (float32r,
    1 cyc/row) -> psum = sg'_t
  - DVE custom ops:  sg''_t = select(u+sg' < th, sg', 0)   [f32r out]
                     u''_t  = select(u+sg' < th, u+sg', 0) [fp32]
  - spike: ACT Sign(u'') as int8 (spike <=> u''==0), decoded host-side.
  - timestep is processed in NSEG psum segments for fine-grained PE/DVE
    overlap (PSUM: NSEG tiles of 8/NSEG banks each).
"""

import os
import numpy as np

import concourse.bacc as bacc
import concourse.mybir as mybir
import concourse.dve_ops as dve_ops
from concourse.dve_spec import Spec, Src0, Src1, C0, Zero, select, lower
from concourse.dve_spec import _has_src1
from concourse.dve_uop import DveOpSpec
from concourse.tile import TileContext
from concourse.bass_utils import run_bass_kernel_spmd

# ---------------- constants -------------------------------------------------
T, B, CIN, H, W = 16, 32, 3, 32, 32
COUT, KK = 128, 3
NCORES = 8
BSH = B // NCORES          # 4 batches per core
CG = np.float32(1.0 - 1.0 / 3.5)
NB = 512                   # positions per psum bank (= one batch half-image)
NPOS = 8 * NB              # 4096 positions per core per timestep
KREPEAT = int(os.environ.get("LIF_KREPEAT", "1"))
NSEG = int(os.environ.get("LIF_NSEG", "4"))    # psum segments per timestep
SEG = NPOS // NSEG                              # positions per segment
BPSEG = SEG // NB                               # batches per segment

# ---------------- custom DVE ops -------------------------------------------
_s = Src0 + Src1


def _register_op(name, spec):
    shas = {}
    for ver in ("v3",):
        uops = lower(spec, ver=ver)
        shas[ver] = DveOpSpec(
            name=name, opcode=0, uops=uops, rd1_en=_has_src1(spec)
        ).sha(ver)
    op = dve_ops.DveOp(name, spec, subdim=False, uops_sha=shas)
    for o in dve_ops.OPS:
        if o.name == name:
            return o
    dve_ops.OPS.append(op)
    dve_ops.CUSTOM_DVE_SPECS[name] = spec
    dve_ops._SUB_OPCODE_FOR_NAME[name] = max(dve_ops._SUB_OPCODE_FOR_NAME.values()) + 1
    assert dve_ops._SUB_OPCODE_FOR_NAME[name] < 0x20
    return op


LIF_U = _register_op(
    "LIF_U",
    Spec(
        body=select(_s < C0, _s, Zero),
        reference=lambda in0, in1, s0, s1, imm2: np.where(
            (in0 + in1) < s0, (in0 + in1).astype(np.float32), 0.0
        ).astype(np.float32),
    ),
)
LIF_SG = _register_op(
    "LIF_SG",
    Spec(
        body=select(_s < C0, Src1, Zero),
        reference=lambda in0, in1, s0, s1, imm2: np.where(
            (in0 + in1) < s0, in1, 0.0
        ).astype(np.float32),
    ),
)
# t=0 state update (u = sg = 0): both new states equal select(ps < th, ps, 0).
LIF_P0 = _register_op(
    "LIF_P0",
    Spec(
        body=select(Src0 < C0, Src0, Zero),
        reference=lambda in0, in1, s0, s1, imm2: np.where(
            in0 < s0, in0, 0.0
        ).astype(np.float32),
    ),
)

# ---------------- device kernel builder -------------------------------------
_NC_CACHE = {}


def _build_nc(krepeat=None):
    krepeat = KREPEAT if krepeat is None else krepeat
    key = (krepeat, NSEG)
    if key in _NC_CACHE:
        return _NC_CACHE[key]
    f32 = mybir.dt.float32
    f32r = mybir.dt.float32r
    nc = bacc.Bacc("TRN2", target_bir_lowering=False)

    xpad = nc.dram_tensor("xpad", [T, COUT, 1156], f32, kind="ExternalInput")
    wmat = nc.dram_tensor("wmat", [COUT, COUT], f32, kind="ExternalInput")
    cgid = nc.dram_tensor("cgid", [COUT, COUT], f32r, kind="ExternalInput")
    th = nc.dram_tensor("th", [COUT, 1], f32, kind="ExternalInput")
    spk = nc.dram_tensor(
        "spk", [T, NSEG, COUT, SEG], mybir.dt.int8, kind="ExternalOutput"
    )

    with TileContext(nc) as tc, \
         tc.tile_pool(name="const", bufs=1) as cpool, \
         tc.tile_pool(name="state", bufs=1) as spool, \
         tc.tile_pool(name="im", bufs=4) as impool, \
         tc.tile_pool(name="out", bufs=2 * NSEG) as opool, \
         tc.tile_pool(name="ps", bufs=NSEG, space="PSUM") as ppool:

        w_sb = cpool.tile([COUT, COUT], f32, tag="w")
        id_sb = cpool.tile([COUT, COUT], f32r, tag="id")
        th_sb = cpool.tile([COUT, 1], f32, tag="th")
        nc.sync.dma_start(w_sb[:], wmat[:])
        nc.sync.dma_start(id_sb[:], cgid[:])
        nc.sync.dma_start(th_sb[:], th[:])

        ubuf = [spool.tile([COUT, NPOS], f32, tag=f"u{i}", name=f"u{i}")
                for i in range(2)]
        gbuf = [spool.tile([COUT, NPOS], f32r, tag=f"g{i}", name=f"g{i}")
                for i in range(2)]
        for _rep in range(krepeat):
          for t in range(T):
              ucur, unext = ubuf[t % 2], ubuf[(t + 1) % 2]
              gcur, gnext = gbuf[t % 2], gbuf[(t + 1) % 2]

              im = impool.tile([COUT, 34, 34], f32, tag="im27")
              eng = nc.sync if t % 2 == 0 else nc.scalar
              eng.dma_start(im[:, :, :], xpad[t, :, :])

              for s in range(NSEG):
                  h, bp = s // (NSEG // 2), s % (NSEG // 2)
                  lo = SEG * s
                  ps = ppool.tile([COUT, SEG], f32, tag="ps")
                  for j in range(BPSEG):
                      b = BPSEG * bp + j
                      nc.tensor.matmul(
                          ps[:, NB * j : NB * (j + 1)],
                          w_sb[32 * b : 32 * b + 27, :],
                          im[32 * b : 32 * b + 27, 16 * h : 16 * h + 16, 0:32],
                          start=True,
                          stop=(t == 0),
                          tile_position=(32 * b, 0),
                          skip_group_check=True,
                      )
                  for j in (range(BPSEG) if t > 0 else []):
                      nc.tensor.matmul(
                          ps[:, NB * j : NB * (j + 1)],
                          id_sb[:],
                          gcur[:, lo + NB * j : lo + NB * (j + 1)],
                          start=False,
                          stop=True,
                          tile_position=(0, 0),
                          skip_group_check=True,
                      )

                  if t == 0:
                      # u = sg = 0: both updates are sel(ps<th, ps, 0).
                      nc.vector._custom_dve(
                          LIF_P0, out=gnext[:, lo : lo + SEG],
                          in0=ps[:], s0=th_sb[:],
                      )
                      nc.vector._custom_dve(
                          LIF_P0, out=unext[:, lo : lo + SEG],
                          in0=ps[:], s0=th_sb[:],
                      )
                  else:
                      if t < T - 1:  # t=T-1 sg state is dead
                          nc.vector._custom_dve(
                              LIF_SG, out=gnext[:, lo : lo + SEG],
                              in0=ucur[:, lo : lo + SEG], in1=ps[:],
                              s0=th_sb[:],
                          )
                      nc.vector._custom_dve(
                          LIF_U, out=unext[:, lo : lo + SEG],
                          in0=ucur[:, lo : lo + SEG], in1=ps[:],
                          s0=th_sb[:],
                      )

                  st = opool.tile([COUT, SEG], mybir.dt.int8, tag="spk")
                  nc.scalar.activation(
                      st[:], unext[:, lo : lo + SEG],
                      mybir.ActivationFunctionType.Sign,
                  )
                  nc.sync.dma_start(spk[t, s, :, :], st[:])

    nc.finalize()
    _NC_CACHE[key] = nc
    return nc


# ---------------- host side --------------------------------------------------
def _prep_weights(weight, norm_weight, norm_bias):
    w = np.clip(weight.astype(np.float32), -4.0, 4.0)
    flat = w.reshape(COUT, -1)
    mean = flat.mean(axis=1, dtype=np.float32)
    var = flat.var(axis=1, ddof=1, dtype=np.float32)
    scale = (norm_weight.reshape(COUT).astype(np.float32)
             / np.sqrt(var + np.float32(1e-5)))
    w_std = (w - mean[:, None, None, None]) * scale[:, None, None, None] \
        + norm_bias.reshape(COUT, 1, 1, 1).astype(np.float32)
    # wmat[32b + 3*(3dy+dx) + c, co] = cg * w_std[co, c, dy, dx]
    wmat = np.zeros((COUT, COUT), np.float32)
    wk = (CG * w_std).transpose(1, 2, 3, 0)  # [c, dy, dx, co]
    for dy in range(3):
        for dx in range(3):
            r = 3 * (3 * dy + dx)
            for b in range(BSH):
                wmat[32 * b + r : 32 * b + r + 3, :] = wk[:, dy, dx, :]
    return wmat


def kernel(x, weight, norm_weight, norm_bias, threshold, _want_trace=False, _krepeat=None):
    x = np.asarray(x, np.float32)
    nc = _build_nc(_krepeat)
    wmat = _prep_weights(np.asarray(weight), np.asarray(norm_weight),
                         np.asarray(norm_bias))
    cgid = (np.eye(COUT) * CG).astype(np.float32)
    th_h = np.asarray(threshold, np.float32).reshape(COUT, 1)

    xp = np.pad(x, [(0, 0), (0, 0), (0, 0), (1, 1), (1, 1)])  # [T,B,C,34,34]
    # x27[t, 32b + 3*(3dy+dx) + c, f] = xpad[t, b, c].flat[34*dy + dx + f]
    xflat = np.pad(xp.reshape(T, B, CIN * 1156), [(0, 0), (0, 0), (0, 128)])
    x27 = np.zeros((T, B, 32, 1156), np.float32)
    for dy in range(3):
        for dx in range(3):
            for c in range(CIN):
                off = c * 1156 + 34 * dy + dx
                x27[:, :, 3 * (3 * dy + dx) + c, :] = xflat[:, :, off : off + 1156]
    in_maps = []
    for core in range(NCORES):
        xs = np.ascontiguousarray(
            x27[:, core * BSH : (core + 1) * BSH].reshape(T, COUT, 1156)
        )
        in_maps.append({"xpad": xs, "wmat": wmat, "cgid": cgid, "th": th_h})

    res = run_bass_kernel_spmd(
        nc, in_maps, core_ids=list(range(NCORES)), trace=_want_trace
    )

    out = np.empty((T, B, COUT, H, W), np.float32)
    for core in range(NCORES):
        s = res.results[core]["spk"]  # [T, NSEG, 128, SEG], spike <=> s == 0
        spikes = (s == 0)
        # segment s=(h, bp): cols = [batch j in pair][row16][w]
        spikes = spikes.reshape(T, 2, NSEG // 2, COUT, BPSEG, 16, W)
        # (t, h, bp, c, j, r, w) -> (t, bp, j, c, h, r, w)
        spikes = spikes.transpose(0, 2, 4, 3, 1, 5, 6)
        out[:, core * BSH : (core + 1) * BSH] = spikes.reshape(
            T, BSH, COUT, H, W
        ).astype(np.float32)
    if _want_trace:
        kernel.last_result = res
    return out


# revision 26
# speedup vs baseline: 1.6695x; 1.0953x over previous
"""Trainium2 Bass kernel for nn_CIFAR10Net LIF conv layer.

Reference computation:
  w' = weight-standardized clip(weight) ; conv2d(x, w', pad=1) over (T*B) frames
  LIF scan over T with state (u, sg) [sm/ss are dead state]:
     sg = (sg + I) * (1 - 1/tau_grad);  u = u + sg
     spike = u >= th ; u, sg *= (1 - spike)
Spikes out: [T, B, 128, 32, 32] f32.

Device mapping (per core, B sharded 4/core over 8 cores):
  - partition dim = Cout (128); free = positions (b, h, w)
  - PE: im2col conv (27-row contraction, 4-way row-packed over b, fp32)
    producing cg*I into PSUM, then accumulates cg*Id @ sg (float32r,
    1 cyc/row) -> psum = sg'_t
  - DVE custom ops:  sg''_t = select(u+sg' < th, sg', 0)   [f32r out]
                     u''_t  = select(u+sg' < th, u+sg', 0) [fp32]
  - spike: ACT Sign(u'') as int8 (spike <=> u''==0), decoded host-side.
  - timestep is processed in NSEG psum segments for fine-grained PE/DVE
    overlap (PSUM: NSEG tiles of 8/NSEG banks each).
"""

import os
import numpy as np

import concourse.bacc as bacc
import concourse.mybir as mybir
import concourse.dve_ops as dve_ops
from concourse.dve_spec import Spec, Src0, Src1, C0, Zero, select, lower
from concourse.dve_spec import _has_src1
from concourse.dve_uop import DveOpSpec
from concourse.tile import TileContext
from concourse.bass_utils import run_bass_kernel_spmd

# ---------------- constants -------------------------------------------------
T, B, CIN, H, W = 16, 32, 3, 32, 32
COUT, KK = 128, 3
NCORES = 8
BSH = B // NCORES          # 4 batches per core
CG = np.float32(1.0 - 1.0 / 3.5)
NB = 512                   # positions per psum bank (= one batch half-image)
NPOS = 8 * NB              # 4096 positions per core per timestep
KREPEAT = int(os.environ.get("LIF_KREPEAT", "1"))
NSEG = int(os.environ.get("LIF_NSEG", "4"))    # psum segments per timestep
SEG = NPOS // NSEG                              # positions per segment
BPSEG = SEG // NB                               # batches per segment

# ---------------- custom DVE ops -------------------------------------------
_s = Src0 + Src1


def _register_op(name, spec):
    shas = {}
    for ver in ("v3",):
        uops = lower(spec, ver=ver)
        shas[ver] = DveOpSpec(
            name=name, opcode=0, uops=uops, rd1_en=_has_src1(spec)
        ).sha(ver)
    op = dve_ops.DveOp(name, spec, subdim=False, uops_sha=shas)
    for o in dve_ops.OPS:
        if o.name == name:
            return o
    dve_ops.OPS.append(op)
    dve_ops.CUSTOM_DVE_SPECS[name] = spec
    dve_ops._SUB_OPCODE_FOR_NAME[name] = max(dve_ops._SUB_OPCODE_FOR_NAME.values()) + 1
    assert dve_ops._SUB_OPCODE_FOR_NAME[name] < 0x20
    return op


LIF_U = _register_op(
    "LIF_U",
    Spec(
        body=select(_s < C0, _s, Zero),
        reference=lambda in0, in1, s0, s1, imm2: np.where(
            (in0 + in1) < s0, (in0 + in1).astype(np.float32), 0.0
        ).astype(np.float32),
    ),
)
LIF_SG = _register_op(
    "LIF_SG",
    Spec(
        body=select(_s < C0, Src1, Zero),
        reference=lambda in0, in1, s0, s1, imm2: np.where(
            (in0 + in1) < s0, in1, 0.0
        ).astype(np.float32),
    ),
)
# t=0 state update (u = sg = 0): both new states equal select(ps < th, ps, 0).
LIF_P0 = _register_op(
    "LIF_P0",
    Spec(
        body=select(Src0 < C0, Src0, Zero),
        reference=lambda in0, in1, s0, s1, imm2: np.where(
            in0 < s0, in0, 0.0
        ).astype(np.float32),
    ),
)

# ---------------- device kernel builder -------------------------------------
_NC_CACHE = {}


def _build_nc(krepeat=None):
    krepeat = KREPEAT if krepeat is None else krepeat
    key = (krepeat, NSEG)
    if key in _NC_CACHE:
        return _NC_CACHE[key]
    f32 = mybir.dt.float32
    f32r = mybir.dt.float32r
    nc = bacc.Bacc("TRN2", target_bir_lowering=False)

    xpad = nc.dram_tensor("xpad", [T, COUT, 1156], f32, kind="ExternalInput")
    wmat = nc.dram_tensor("wmat", [COUT, COUT], f32, kind="ExternalInput")
    cgid = nc.dram_tensor("cgid", [COUT, COUT], f32r, kind="ExternalInput")
    th = nc.dram_tensor("th", [COUT, 1], f32, kind="ExternalInput")
    spk = nc.dram_tensor(
        "spk", [T, NSEG, COUT, SEG], mybir.dt.int8, kind="ExternalOutput"
    )

    with TileContext(nc) as tc, \
         tc.tile_pool(name="const", bufs=1) as cpool, \
         tc.tile_pool(name="state", bufs=1) as spool, \
         tc.tile_pool(name="im", bufs=4) as impool, \
         tc.tile_pool(name="out", bufs=2 * NSEG) as opool, \
         tc.tile_pool(name="ps", bufs=NSEG, space="PSUM") as ppool:

        w_sb = cpool.tile([COUT, COUT], f32, tag="w")
        id_sb = cpool.tile([COUT, COUT], f32r, tag="id")
        th_sb = cpool.tile([COUT, 1], f32, tag="th")
        # Spread init DMAs over idle engine queues; sync's first DMA must be
        # im(t=0) so the conv pipeline starts ASAP.
        nc.scalar.dma_start(w_sb[:], wmat[:])
        nc.gpsimd.dma_start(id_sb[:], cgid[:])
        nc.gpsimd.dma_start(th_sb[:], th[:])

        ubuf = [spool.tile([COUT, NPOS], f32, tag=f"u{i}", name=f"u{i}")
                for i in range(2)]
        gbuf = [spool.tile([COUT, NPOS], f32r, tag=f"g{i}", name=f"g{i}")
                for i in range(2)]

        # Ramp the PE to full pstate during the initial DMA lead-in: the
        # first real convs otherwise run 2x slow (pipeline warmup is ~3us of
        # continuous PE busy). Dummy f32r matmuls on zeroed scratch.
        warm = cpool.tile([COUT, NB], f32r, tag="warm")
        nc.vector.memset(warm[:].bitcast(f32), 0.0)
        warm_ps = ppool.tile([COUT, SEG], f32, tag="ps")
        for _ in range(8):
            nc.tensor.matmul(
                warm_ps[:, 0:NB], warm[:, 0:COUT], warm[:],
                start=True, stop=True, skip_group_check=True,
            )

        for _rep in range(krepeat):
          for t in range(T):
              ucur, unext = ubuf[t % 2], ubuf[(t + 1) % 2]
              gcur, gnext = gbuf[t % 2], gbuf[(t + 1) % 2]

              im = impool.tile([COUT, 34, 34], f32, tag="im27")
              if t == 0:
                  # split the first load across two queues to cut the lead-in
                  nc.sync.dma_start(im[:, 0:17, :], xpad[t, :, 0:578])
                  nc.scalar.dma_start(im[:, 17:34, :], xpad[t, :, 578:1156])
              else:
                  eng = nc.sync if t % 2 == 0 else nc.scalar
                  eng.dma_start(im[:, :, :], xpad[t, :, :])

              for s in range(NSEG):
                  h, bp = s // (NSEG // 2), s % (NSEG // 2)
                  lo = SEG * s
                  ps = ppool.tile([COUT, SEG], f32, tag="ps")
                  for j in range(BPSEG):
                      b = BPSEG * bp + j
                      nc.tensor.matmul(
                          ps[:, NB * j : NB * (j + 1)],
                          w_sb[32 * b : 32 * b + 27, :],
                          im[32 * b : 32 * b + 27, 16 * h : 16 * h + 16, 0:32],
                          start=True,
                          stop=(t == 0),
                          tile_position=(32 * b, 0),
                          skip_group_check=True,
                      )
                  for j in (range(BPSEG) if t > 0 else []):
                      nc.tensor.matmul(
                          ps[:, NB * j : NB * (j + 1)],
                          id_sb[:],
                          gcur[:, lo + NB * j : lo + NB * (j + 1)],
                          start=False,
                          stop=True,
                          tile_position=(0, 0),
                          skip_group_check=True,
                      )

                  if t == 0:
                      # u = sg = 0: both updates are sel(ps<th, ps, 0).
                      nc.vector._custom_dve(
                          LIF_P0, out=gnext[:, lo : lo + SEG],
                          in0=ps[:], s0=th_sb[:],
                      )
                      nc.vector._custom_dve(
                          LIF_P0, out=unext[:, lo : lo + SEG],
                          in0=ps[:], s0=th_sb[:],
                      )
                  else:
                      if t < T - 1:  # t=T-1 sg state is dead
                          nc.vector._custom_dve(
                              LIF_SG, out=gnext[:, lo : lo + SEG],
                              in0=ucur[:, lo : lo + SEG], in1=ps[:],
                              s0=th_sb[:],
                          )
                      nc.vector._custom_dve(
                          LIF_U, out=unext[:, lo : lo + SEG],
                          in0=ucur[:, lo : lo + SEG], in1=ps[:],
                          s0=th_sb[:],
                      )

                  st = opool.tile([COUT, SEG], mybir.dt.int8, tag="spk")
                  nc.scalar.activation(
                      st[:], unext[:, lo : lo + SEG],
                      mybir.ActivationFunctionType.Sign,
                  )
                  nc.sync.dma_start(spk[t, s, :, :], st[:])

    nc.finalize()
    _NC_CACHE[key] = nc
    return nc


# ---------------- host side --------------------------------------------------
def _prep_weights(weight, norm_weight, norm_bias):
    w = np.clip(weight.astype(np.float32), -4.0, 4.0)
    flat = w.reshape(COUT, -1)
    mean = flat.mean(axis=1, dtype=np.float32)
    var = flat.var(axis=1, ddof=1, dtype=np.float32)
    scale = (norm_weight.reshape(COUT).astype(np.float32)
             / np.sqrt(var + np.float32(1e-5)))
    w_std = (w - mean[:, None, None, None]) * scale[:, None, None, None] \
        + norm_bias.reshape(COUT, 1, 1, 1).astype(np.float32)
    # wmat[32b + 3*(3dy+dx) + c, co] = cg * w_std[co, c, dy, dx]
    wmat = np.zeros((COUT, COUT), np.float32)
    wk = (CG * w_std).transpose(1, 2, 3, 0)  # [c, dy, dx, co]
    for dy in range(3):
        for dx in range(3):
            r = 3 * (3 * dy + dx)
            for b in range(BSH):
                wmat[32 * b + r : 32 * b + r + 3, :] = wk[:, dy, dx, :]
    return wmat


def kernel(x, weight, norm_weight, norm_bias, threshold, _want_trace=False, _krepeat=None):
    x = np.asarray(x, np.float32)
    nc = _build_nc(_krepeat)
    wmat = _prep_weights(np.asarray(weight), np.asarray(norm_weight),
                         np.asarray(norm_bias))
    cgid = (np.eye(COUT) * CG).astype(np.float32)
    th_h = np.asarray(threshold, np.float32).reshape(COUT, 1)

    xp = np.pad(x, [(0, 0), (0, 0), (0, 0), (1, 1), (1, 1)])  # [T,B,C,34,34]
    # x27[t, 32b + 3*(3dy+dx) + c, f] = xpad[t, b, c].flat[34*dy + dx + f]
    xflat = np.pad(xp.reshape(T, B, CIN * 1156), [(0, 0), (0, 0), (0, 128)])
    x27 = np.zeros((T, B, 32, 1156), np.float32)
    for dy in range(3):
        for dx in range(3):
            for c in range(CIN):
                off = c * 1156 + 34 * dy + dx
                x27[:, :, 3 * (3 * dy + dx) + c, :] = xflat[:, :, off : off + 1156]
    in_maps = []
    for core in range(NCORES):
        xs = np.ascontiguousarray(
            x27[:, core * BSH : (core + 1) * BSH].reshape(T, COUT, 1156)
        )
        in_maps.append({"xpad": xs, "wmat": wmat, "cgid": cgid, "th": th_h})

    res = run_bass_kernel_spmd(
        nc, in_maps, core_ids=list(range(NCORES)), trace=_want_trace
    )

    out = np.empty((T, B, COUT, H, W), np.float32)
    for core in range(NCORES):
        s = res.results[core]["spk"]  # [T, NSEG, 128, SEG], spike <=> s == 0
        spikes = (s == 0)
        # segment s=(h, bp): cols = [batch j in pair][row16][w]
        spikes = spikes.reshape(T, 2, NSEG // 2, COUT, BPSEG, 16, W)
        # (t, h, bp, c, j, r, w) -> (t, bp, j, c, h, r, w)
        spikes = spikes.transpose(0, 2, 4, 3, 1, 5, 6)
        out[:, core * BSH : (core + 1) * BSH] = spikes.reshape(
            T, BSH, COUT, H, W
        ).astype(np.float32)
    if _want_trace:
        kernel.last_result = res
    return out


# revision 30
# speedup vs baseline: 1.6769x; 1.0045x over previous
"""Trainium2 Bass kernel for nn_CIFAR10Net LIF conv layer.

Reference computation:
  w' = weight-standardized clip(weight) ; conv2d(x, w', pad=1) over (T*B) frames
  LIF scan over T with state (u, sg) [sm/ss are dead state]:
     sg = (sg + I) * (1 - 1/tau_grad);  u = u + sg
     spike = u >= th ; u, sg *= (1 - spike)
Spikes out: [T, B, 128, 32, 32] f32.

Device mapping (per core, B sharded 4/core over 8 cores):
  - partition dim = Cout (128); free = positions (b, h, w)
  - PE: im2col conv (27-row contraction, 4-way row-packed over b, fp32)
    producing cg*I into PSUM, then accumulates cg*Id @ sg (float32r,
    1 cyc/row) -> psum = sg'_t
  - DVE custom ops:  sg''_t = select(u+sg' < th, sg', 0)   [f32r out]
                     u''_t  = select(u+sg' < th, u+sg', 0) [fp32]
  - spike: ACT Sign(u'') as int8 (spike <=> u''==0), decoded host-side.
  - timestep is processed in NSEG psum segments for fine-grained PE/DVE
    overlap (PSUM: NSEG tiles of 8/NSEG banks each).
"""

import os
import numpy as np

import concourse.bacc as bacc
import concourse.mybir as mybir
import concourse.dve_ops as dve_ops
from concourse.dve_spec import Spec, Src0, Src1, C0, Zero, select, lower
from concourse.dve_spec import _has_src1
from concourse.dve_uop import DveOpSpec
from concourse.tile import TileContext
from concourse.bass_utils import run_bass_kernel_spmd

# ---------------- constants -------------------------------------------------
T, B, CIN, H, W = 16, 32, 3, 32, 32
COUT, KK = 128, 3
NCORES = 8
BSH = B // NCORES          # 4 batches per core
CG = np.float32(1.0 - 1.0 / 3.5)
NB = 512                   # positions per psum bank (= one batch half-image)
NPOS = 8 * NB              # 4096 positions per core per timestep
KREPEAT = int(os.environ.get("LIF_KREPEAT", "1"))
NSEG = int(os.environ.get("LIF_NSEG", "4"))    # psum segments per timestep
NWARM = int(os.environ.get("LIF_NWARM", "4"))  # PE pstate-ramp dummy matmuls
SEG = NPOS // NSEG                              # positions per segment
BPSEG = SEG // NB                               # batches per segment

# ---------------- custom DVE ops -------------------------------------------
_s = Src0 + Src1


def _register_op(name, spec):
    shas = {}
    for ver in ("v3",):
        uops = lower(spec, ver=ver)
        shas[ver] = DveOpSpec(
            name=name, opcode=0, uops=uops, rd1_en=_has_src1(spec)
        ).sha(ver)
    op = dve_ops.DveOp(name, spec, subdim=False, uops_sha=shas)
    for o in dve_ops.OPS:
        if o.name == name:
            return o
    dve_ops.OPS.append(op)
    dve_ops.CUSTOM_DVE_SPECS[name] = spec
    dve_ops._SUB_OPCODE_FOR_NAME[name] = max(dve_ops._SUB_OPCODE_FOR_NAME.values()) + 1
    assert dve_ops._SUB_OPCODE_FOR_NAME[name] < 0x20
    return op


LIF_U = _register_op(
    "LIF_U",
    Spec(
        body=select(_s < C0, _s, Zero),
        reference=lambda in0, in1, s0, s1, imm2: np.where(
            (in0 + in1) < s0, (in0 + in1).astype(np.float32), 0.0
        ).astype(np.float32),
    ),
)
LIF_SG = _register_op(
    "LIF_SG",
    Spec(
        body=select(_s < C0, Src1, Zero),
        reference=lambda in0, in1, s0, s1, imm2: np.where(
            (in0 + in1) < s0, in1, 0.0
        ).astype(np.float32),
    ),
)
# t=0 state update (u = sg = 0): both new states equal select(ps < th, ps, 0).
LIF_P0 = _register_op(
    "LIF_P0",
    Spec(
        body=select(Src0 < C0, Src0, Zero),
        reference=lambda in0, in1, s0, s1, imm2: np.where(
            in0 < s0, in0, 0.0
        ).astype(np.float32),
    ),
)

# ---------------- device kernel builder -------------------------------------
_NC_CACHE = {}


def _build_nc(krepeat=None):
    krepeat = KREPEAT if krepeat is None else krepeat
    key = (krepeat, NSEG, NWARM)
    if key in _NC_CACHE:
        return _NC_CACHE[key]
    f32 = mybir.dt.float32
    f32r = mybir.dt.float32r
    nc = bacc.Bacc("TRN2", target_bir_lowering=False)

    xpad = nc.dram_tensor("xpad", [T, COUT, 1156], f32, kind="ExternalInput")
    wmat = nc.dram_tensor("wmat", [COUT, COUT], f32, kind="ExternalInput")
    cgid = nc.dram_tensor("cgid", [COUT, COUT], f32r, kind="ExternalInput")
    th = nc.dram_tensor("th", [COUT, 1], f32, kind="ExternalInput")
    spk = nc.dram_tensor(
        "spk", [T, NSEG, COUT, SEG], mybir.dt.int8, kind="ExternalOutput"
    )

    with TileContext(nc) as tc, \
         tc.tile_pool(name="const", bufs=1) as cpool, \
         tc.tile_pool(name="state", bufs=1) as spool, \
         tc.tile_pool(name="im", bufs=4) as impool, \
         tc.tile_pool(name="out", bufs=2 * NSEG) as opool, \
         tc.tile_pool(name="ps", bufs=NSEG, space="PSUM") as ppool:

        w_sb = cpool.tile([COUT, COUT], f32, tag="w")
        id_sb = cpool.tile([COUT, COUT], f32r, tag="id")
        th_sb = cpool.tile([COUT, 1], f32, tag="th")
        # Spread init DMAs over idle engine queues; sync's first DMA must be
        # im(t=0) so the conv pipeline starts ASAP.
        nc.scalar.dma_start(w_sb[:], wmat[:])
        nc.gpsimd.dma_start(id_sb[:], cgid[:])
        nc.gpsimd.dma_start(th_sb[:], th[:])

        ubuf = [spool.tile([COUT, NPOS], f32, tag=f"u{i}", name=f"u{i}")
                for i in range(2)]
        gbuf = [spool.tile([COUT, NPOS], f32r, tag=f"g{i}", name=f"g{i}")
                for i in range(2)]

        # Ramp the PE to full pstate during the initial DMA lead-in: the
        # first real convs otherwise run 2x slow (pipeline warmup is ~3us of
        # continuous PE busy). Dummy f32r matmuls on zeroed scratch.
        warm = cpool.tile([COUT, NB], f32r, tag="warm")
        nc.vector.memset(warm[:].bitcast(f32), 0.0)
        warm_ps = ppool.tile([COUT, SEG], f32, tag="ps")
        for _ in range(NWARM):
            nc.tensor.matmul(
                warm_ps[:, 0:NB], warm[:, 0:COUT], warm[:],
                start=True, stop=True, skip_group_check=True,
            )

        for _rep in range(krepeat):
          for t in range(T):
              ucur, unext = ubuf[t % 2], ubuf[(t + 1) % 2]
              gcur, gnext = gbuf[t % 2], gbuf[(t + 1) % 2]

              if t == 0:
                  # Two separate tiles on two queues: conv(h=0) starts as soon
                  # as the first chunk lands, without waiting for the second.
                  im0 = impool.tile([COUT, 18, 34], f32, tag="im0")
                  im1 = impool.tile([COUT, 18, 34], f32, tag="im1")
                  nc.sync.dma_start(im0[:, :, :], xpad[t, :, 0:612])
                  nc.scalar.dma_start(im1[:, :, :], xpad[t, :, 544:1156])
              else:
                  im = impool.tile([COUT, 34, 34], f32, tag="im27")
                  eng = nc.sync if t % 2 == 0 else nc.scalar
                  eng.dma_start(im[:, :, :], xpad[t, :, :])

              for s in range(NSEG):
                  h, bp = s // (NSEG // 2), s % (NSEG // 2)
                  lo = SEG * s
                  ps = ppool.tile([COUT, SEG], f32, tag="ps")
                  if t == 0:
                      imsrc, imrow = (im0, 0) if h == 0 else (im1, 0)
                  else:
                      imsrc, imrow = im, 16 * h
                  for j in range(BPSEG):
                      b = BPSEG * bp + j
                      nc.tensor.matmul(
                          ps[:, NB * j : NB * (j + 1)],
                          w_sb[32 * b : 32 * b + 27, :],
                          imsrc[32 * b : 32 * b + 27, imrow : imrow + 16, 0:32],
                          start=True,
                          stop=(t == 0),
                          tile_position=(32 * b, 0),
                          skip_group_check=True,
                      )
                  for j in (range(BPSEG) if t > 0 else []):
                      nc.tensor.matmul(
                          ps[:, NB * j : NB * (j + 1)],
                          id_sb[:],
                          gcur[:, lo + NB * j : lo + NB * (j + 1)],
                          start=False,
                          stop=True,
                          tile_position=(0, 0),
                          skip_group_check=True,
                      )

                  if t == 0:
                      # u = sg = 0: both updates are sel(ps<th, ps, 0).
                      nc.vector._custom_dve(
                          LIF_P0, out=gnext[:, lo : lo + SEG],
                          in0=ps[:], s0=th_sb[:],
                      )
                      nc.vector._custom_dve(
                          LIF_P0, out=unext[:, lo : lo + SEG],
                          in0=ps[:], s0=th_sb[:],
                      )
                  else:
                      if t < T - 1:  # t=T-1 sg state is dead
                          nc.vector._custom_dve(
                              LIF_SG, out=gnext[:, lo : lo + SEG],
                              in0=ucur[:, lo : lo + SEG], in1=ps[:],
                              s0=th_sb[:],
                          )
                      nc.vector._custom_dve(
                          LIF_U, out=unext[:, lo : lo + SEG],
                          in0=ucur[:, lo : lo + SEG], in1=ps[:],
                          s0=th_sb[:],
                      )

                  st = opool.tile([COUT, SEG], mybir.dt.int8, tag="spk")
                  nc.scalar.activation(
                      st[:], unext[:, lo : lo + SEG],
                      mybir.ActivationFunctionType.Sign,
                  )
                  nc.sync.dma_start(spk[t, s, :, :], st[:])

    nc.finalize()
    _NC_CACHE[key] = nc
    return nc


# ---------------- host side --------------------------------------------------
def _prep_weights(weight, norm_weight, norm_bias):
    w = np.clip(weight.astype(np.float32), -4.0, 4.0)
    flat = w.reshape(COUT, -1)
    mean = flat.mean(axis=1, dtype=np.float32)
    var = flat.var(axis=1, ddof=1, dtype=np.float32)
    scale = (norm_weight.reshape(COUT).astype(np.float32)
             / np.sqrt(var + np.float32(1e-5)))
    w_std = (w - mean[:, None, None, None]) * scale[:, None, None, None] \
        + norm_bias.reshape(COUT, 1, 1, 1).astype(np.float32)
    # wmat[32b + 3*(3dy+dx) + c, co] = cg * w_std[co, c, dy, dx]
    wmat = np.zeros((COUT, COUT), np.float32)
    wk = (CG * w_std).transpose(1, 2, 3, 0)  # [c, dy, dx, co]
    for dy in range(3):
        for dx in range(3):
            r = 3 * (3 * dy + dx)
            for b in range(BSH):
                wmat[32 * b + r : 32 * b + r + 3, :] = wk[:, dy, dx, :]
    return wmat


def kernel(x, weight, norm_weight, norm_bias, threshold, _want_trace=False, _krepeat=None):
    x = np.asarray(x, np.float32)
    nc = _build_nc(_krepeat)
    wmat = _prep_weights(np.asarray(weight), np.asarray(norm_weight),
                         np.asarray(norm_bias))
    cgid = (np.eye(COUT) * CG).astype(np.float32)
    th_h = np.asarray(threshold, np.float32).reshape(COUT, 1)

    xp = np.pad(x, [(0, 0), (0, 0), (0, 0), (1, 1), (1, 1)])  # [T,B,C,34,34]
    # x27[t, 32b + 3*(3dy+dx) + c, f] = xpad[t, b, c].flat[34*dy + dx + f]
    xflat = np.pad(xp.reshape(T, B, CIN * 1156), [(0, 0), (0, 0), (0, 128)])
    x27 = np.zeros((T, B, 32, 1156), np.float32)
    for dy in range(3):
        for dx in range(3):
            for c in range(CIN):
                off = c * 1156 + 34 * dy + dx
                x27[:, :, 3 * (3 * dy + dx) + c, :] = xflat[:, :, off : off + 1156]
    in_maps = []
    for core in range(NCORES):
        xs = np.ascontiguousarray(
            x27[:, core * BSH : (core + 1) * BSH].reshape(T, COUT, 1156)
        )
        in_maps.append({"xpad": xs, "wmat": wmat, "cgid": cgid, "th": th_h})

    res = run_bass_kernel_spmd(
        nc, in_maps, core_ids=list(range(NCORES)), trace=_want_trace
    )

    out = np.empty((T, B, COUT, H, W), np.float32)
    for core in range(NCORES):
        s = res.results[core]["spk"]  # [T, NSEG, 128, SEG], spike <=> s == 0
        spikes = (s == 0)
        # segment s=(h, bp): cols = [batch j in pair][row16][w]
        spikes = spikes.reshape(T, 2, NSEG // 2, COUT, BPSEG, 16, W)
        # (t, h, bp, c, j, r, w) -> (t, bp, j, c, h, r, w)
        spikes = spikes.transpose(0, 2, 4, 3, 1, 5, 6)
        out[:, core * BSH : (core + 1) * BSH] = spikes.reshape(
            T, BSH, COUT, H, W
        ).astype(np.float32)
    if _want_trace:
        kernel.last_result = res
    return out


# revision 31
# speedup vs baseline: 1.6985x; 1.0129x over previous
"""Trainium2 Bass kernel for nn_CIFAR10Net LIF conv layer.

Reference computation:
  w' = weight-standardized clip(weight) ; conv2d(x, w', pad=1) over (T*B) frames
  LIF scan over T with state (u, sg) [sm/ss are dead state]:
     sg = (sg + I) * (1 - 1/tau_grad);  u = u + sg
     spike = u >= th ; u, sg *= (1 - spike)
Spikes out: [T, B, 128, 32, 32] f32.

Device mapping (per core, B sharded 4/core over 8 cores):
  - partition dim = Cout (128); free = positions (b, h, w)
  - PE: im2col conv (27-row contraction, 4-way row-packed over b, fp32)
    producing cg*I into PSUM, then accumulates cg*Id @ sg (float32r,
    1 cyc/row) -> psum = sg'_t
  - DVE custom ops:  sg''_t = select(u+sg' < th, sg', 0)   [f32r out]
                     u''_t  = select(u+sg' < th, u+sg', 0) [fp32]
  - spike: ACT Sign(u'') as int8 (spike <=> u''==0), decoded host-side.
  - timestep is processed in NSEG psum segments for fine-grained PE/DVE
    overlap (PSUM: NSEG tiles of 8/NSEG banks each).
"""

import os
import numpy as np

import concourse.bacc as bacc
import concourse.mybir as mybir
import concourse.dve_ops as dve_ops
from concourse.dve_spec import Spec, Src0, Src1, C0, Zero, select, lower
from concourse.dve_spec import _has_src1
from concourse.dve_uop import DveOpSpec
from concourse.tile import TileContext
from concourse.bass_utils import run_bass_kernel_spmd

# ---------------- constants -------------------------------------------------
T, B, CIN, H, W = 16, 32, 3, 32, 32
COUT, KK = 128, 3
NCORES = 8
BSH = B // NCORES          # 4 batches per core
CG = np.float32(1.0 - 1.0 / 3.5)
NB = 512                   # positions per psum bank (= one batch half-image)
NPOS = 8 * NB              # 4096 positions per core per timestep
KREPEAT = int(os.environ.get("LIF_KREPEAT", "1"))
NSEG = int(os.environ.get("LIF_NSEG", "4"))    # psum segments per timestep
NWARM = int(os.environ.get("LIF_NWARM", "4"))  # PE pstate-ramp dummy matmuls
SEG = NPOS // NSEG                              # positions per segment
BPSEG = SEG // NB                               # batches per segment

# ---------------- custom DVE ops -------------------------------------------
_s = Src0 + Src1


def _register_op(name, spec):
    shas = {}
    for ver in ("v3",):
        uops = lower(spec, ver=ver)
        shas[ver] = DveOpSpec(
            name=name, opcode=0, uops=uops, rd1_en=_has_src1(spec)
        ).sha(ver)
    op = dve_ops.DveOp(name, spec, subdim=False, uops_sha=shas)
    for o in dve_ops.OPS:
        if o.name == name:
            return o
    dve_ops.OPS.append(op)
    dve_ops.CUSTOM_DVE_SPECS[name] = spec
    dve_ops._SUB_OPCODE_FOR_NAME[name] = max(dve_ops._SUB_OPCODE_FOR_NAME.values()) + 1
    assert dve_ops._SUB_OPCODE_FOR_NAME[name] < 0x20
    return op


LIF_U = _register_op(
    "LIF_U",
    Spec(
        body=select(_s < C0, _s, Zero),
        reference=lambda in0, in1, s0, s1, imm2: np.where(
            (in0 + in1) < s0, (in0 + in1).astype(np.float32), 0.0
        ).astype(np.float32),
    ),
)
LIF_SG = _register_op(
    "LIF_SG",
    Spec(
        body=select(_s < C0, Src1, Zero),
        reference=lambda in0, in1, s0, s1, imm2: np.where(
            (in0 + in1) < s0, in1, 0.0
        ).astype(np.float32),
    ),
)
# t=0 state update (u = sg = 0): both new states equal select(ps < th, ps, 0).
LIF_P0 = _register_op(
    "LIF_P0",
    Spec(
        body=select(Src0 < C0, Src0, Zero),
        reference=lambda in0, in1, s0, s1, imm2: np.where(
            in0 < s0, in0, 0.0
        ).astype(np.float32),
    ),
)

# ---------------- device kernel builder -------------------------------------
_NC_CACHE = {}


def _build_nc(krepeat=None):
    krepeat = KREPEAT if krepeat is None else krepeat
    key = (krepeat, NSEG, NWARM)
    if key in _NC_CACHE:
        return _NC_CACHE[key]
    f32 = mybir.dt.float32
    f32r = mybir.dt.float32r
    nc = bacc.Bacc("TRN2", target_bir_lowering=False)

    xpad = nc.dram_tensor("xpad", [T, COUT, 1156], f32, kind="ExternalInput")
    wmat = nc.dram_tensor("wmat", [COUT, COUT], f32, kind="ExternalInput")
    cgid = nc.dram_tensor("cgid", [COUT, COUT], f32r, kind="ExternalInput")
    th = nc.dram_tensor("th", [COUT, 1], f32, kind="ExternalInput")
    spk = nc.dram_tensor(
        "spk", [T, NSEG, COUT, SEG], mybir.dt.int8, kind="ExternalOutput"
    )

    with TileContext(nc) as tc, \
         tc.tile_pool(name="const", bufs=1) as cpool, \
         tc.tile_pool(name="state", bufs=1) as spool, \
         tc.tile_pool(name="im", bufs=4) as impool, \
         tc.tile_pool(name="out", bufs=2 * NSEG) as opool, \
         tc.tile_pool(name="ps", bufs=NSEG, space="PSUM") as ppool:

        w_sb = cpool.tile([COUT, COUT], f32, tag="w")
        id_sb = cpool.tile([COUT, COUT], f32r, tag="id")
        th_sb = cpool.tile([COUT, 1], f32, tag="th")
        # Spread init DMAs over idle engine queues; sync's first DMA must be
        # im(t=0) so the conv pipeline starts ASAP.
        nc.scalar.dma_start(w_sb[:], wmat[:])
        nc.gpsimd.dma_start(id_sb[:], cgid[:])
        nc.gpsimd.dma_start(th_sb[:], th[:])

        ubuf = [spool.tile([COUT, NPOS], f32, tag=f"u{i}", name=f"u{i}")
                for i in range(2)]
        gbuf = [spool.tile([COUT, NPOS], f32r, tag=f"g{i}", name=f"g{i}")
                for i in range(2)]

        # Ramp the PE to full pstate during the initial DMA lead-in: the
        # first real convs otherwise run 2x slow (pipeline warmup is ~3us of
        # continuous PE busy). Dummy f32r matmuls on zeroed scratch.
        warm = cpool.tile([COUT, NB], f32r, tag="warm")
        nc.vector.memset(warm[:].bitcast(f32), 0.0)
        warm_ps = ppool.tile([COUT, SEG], f32, tag="ps")
        for _ in range(NWARM):
            nc.tensor.matmul(
                warm_ps[:, 0:NB], warm[:, 0:COUT], warm[:],
                start=True, stop=True, skip_group_check=True,
            )

        for _rep in range(krepeat):
          for t in range(T):
              ucur, unext = ubuf[t % 2], ubuf[(t + 1) % 2]
              gcur, gnext = gbuf[t % 2], gbuf[(t + 1) % 2]

              if t == 0:
                  # Two separate tiles on two queues: conv(h=0) starts as soon
                  # as the first chunk lands, without waiting for the second.
                  im0 = impool.tile([COUT, 18, 34], f32, tag="im0")
                  im1 = impool.tile([COUT, 18, 34], f32, tag="im1")
                  nc.sync.dma_start(im0[:, :, :], xpad[t, :, 0:612])
                  nc.scalar.dma_start(im1[:, :, :], xpad[t, :, 544:1156])
              else:
                  im = impool.tile([COUT, 34, 34], f32, tag="im27")
                  eng = nc.sync if t % 2 == 0 else nc.scalar
                  eng.dma_start(im[:, :, :], xpad[t, :, :])

              for s in range(NSEG):
                  h, bp = s // (NSEG // 2), s % (NSEG // 2)
                  lo = SEG * s
                  ps = ppool.tile([COUT, SEG], f32, tag="ps")
                  if t == 0:
                      imsrc, imrow = (im0, 0) if h == 0 else (im1, 0)
                  else:
                      imsrc, imrow = im, 16 * h
                  for j in range(BPSEG):
                      b = BPSEG * bp + j
                      nc.tensor.matmul(
                          ps[:, NB * j : NB * (j + 1)],
                          w_sb[32 * b : 32 * b + 27, :],
                          imsrc[32 * b : 32 * b + 27, imrow : imrow + 16, 0:32],
                          start=True,
                          stop=(t == 0),
                          tile_position=(32 * b, 0),
                          skip_group_check=True,
                      )
                  for j in (range(BPSEG) if t > 0 else []):
                      nc.tensor.matmul(
                          ps[:, NB * j : NB * (j + 1)],
                          id_sb[:],
                          gcur[:, lo + NB * j : lo + NB * (j + 1)],
                          start=False,
                          stop=True,
                          tile_position=(0, 0),
                          skip_group_check=True,
                      )

                  if t == 0:
                      # u = sg = 0: both updates are sel(ps<th, ps, 0); write
                      # the f32r copy only, t=1 reads it as both u and sg.
                      nc.vector._custom_dve(
                          LIF_P0, out=gnext[:, lo : lo + SEG],
                          in0=ps[:], s0=th_sb[:],
                      )
                      spike_src = gnext[:, lo : lo + SEG].bitcast(f32)
                  else:
                      u_in = (gcur if t == 1 else ucur)[:, lo : lo + SEG]
                      u_in = u_in.bitcast(f32) if t == 1 else u_in
                      if t < T - 1:  # t=T-1 sg state is dead
                          nc.vector._custom_dve(
                              LIF_SG, out=gnext[:, lo : lo + SEG],
                              in0=u_in, in1=ps[:],
                              s0=th_sb[:],
                          )
                      nc.vector._custom_dve(
                          LIF_U, out=unext[:, lo : lo + SEG],
                          in0=u_in, in1=ps[:],
                          s0=th_sb[:],
                      )
                      spike_src = unext[:, lo : lo + SEG]

                  st = opool.tile([COUT, SEG], mybir.dt.int8, tag="spk")
                  nc.scalar.activation(
                      st[:], spike_src, mybir.ActivationFunctionType.Sign,
                  )
                  nc.sync.dma_start(spk[t, s, :, :], st[:])

    nc.finalize()
    _NC_CACHE[key] = nc
    return nc


# ---------------- host side --------------------------------------------------
def _prep_weights(weight, norm_weight, norm_bias):
    w = np.clip(weight.astype(np.float32), -4.0, 4.0)
    flat = w.reshape(COUT, -1)
    mean = flat.mean(axis=1, dtype=np.float32)
    var = flat.var(axis=1, ddof=1, dtype=np.float32)
    scale = (norm_weight.reshape(COUT).astype(np.float32)
             / np.sqrt(var + np.float32(1e-5)))
    w_std = (w - mean[:, None, None, None]) * scale[:, None, None, None] \
        + norm_bias.reshape(COUT, 1, 1, 1).astype(np.float32)
    # wmat[32b + 3*(3dy+dx) + c, co] = cg * w_std[co, c, dy, dx]
    wmat = np.zeros((COUT, COUT), np.float32)
    wk = (CG * w_std).transpose(1, 2, 3, 0)  # [c, dy, dx, co]
    for dy in range(3):
        for dx in range(3):
            r = 3 * (3 * dy + dx)
            for b in range(BSH):
                wmat[32 * b + r : 32 * b + r + 3, :] = wk[:, dy, dx, :]
    return wmat


def kernel(x, weight, norm_weight, norm_bias, threshold, _want_trace=False, _krepeat=None):
    x = np.asarray(x, np.float32)
    nc = _build_nc(_krepeat)
    wmat = _prep_weights(np.asarray(weight), np.asarray(norm_weight),
                         np.asarray(norm_bias))
    cgid = (np.eye(COUT) * CG).astype(np.float32)
    th_h = np.asarray(threshold, np.float32).reshape(COUT, 1)

    xp = np.pad(x, [(0, 0), (0, 0), (0, 0), (1, 1), (1, 1)])  # [T,B,C,34,34]
    # x27[t, 32b + 3*(3dy+dx) + c, f] = xpad[t, b, c].flat[34*dy + dx + f]
    xflat = np.pad(xp.reshape(T, B, CIN * 1156), [(0, 0), (0, 0), (0, 128)])
    x27 = np.zeros((T, B, 32, 1156), np.float32)
    for dy in range(3):
        for dx in range(3):
            for c in range(CIN):
                off = c * 1156 + 34 * dy + dx
                x27[:, :, 3 * (3 * dy + dx) + c, :] = xflat[:, :, off : off + 1156]
    in_maps = []
    for core in range(NCORES):
        xs = np.ascontiguousarray(
            x27[:, core * BSH : (core + 1) * BSH].reshape(T, COUT, 1156)
        )
        in_maps.append({"xpad": xs, "wmat": wmat, "cgid": cgid, "th": th_h})

    res = run_bass_kernel_spmd(
        nc, in_maps, core_ids=list(range(NCORES)), trace=_want_trace
    )

    out = np.empty((T, B, COUT, H, W), np.float32)
    for core in range(NCORES):
        s = res.results[core]["spk"]  # [T, NSEG, 128, SEG], spike <=> s == 0
        spikes = (s == 0)
        # segment s=(h, bp): cols = [batch j in pair][row16][w]
        spikes = spikes.reshape(T, 2, NSEG // 2, COUT, BPSEG, 16, W)
        # (t, h, bp, c, j, r, w) -> (t, bp, j, c, h, r, w)
        spikes = spikes.transpose(0, 2, 4, 3, 1, 5, 6)
        out[:, core * BSH : (core + 1) * BSH] = spikes.reshape(
            T, BSH, COUT, H, W
        ).astype(np.float32)
    if _want_trace:
        kernel.last_result = res
    return out
